# revision 1
# baseline (speedup 1.0000x reference)
"""Differential multi-head attention on 8 TRN2 NeuronCores (Bass/Tile).

Problem (hardcoded): B=2, T=N=2048, HID=1024, H=16 heads, DH=64, HALF=32,
DEPTH=6, causal. Reference:
    q = query @ Wq.T ; k = key_ @ Wk.T ; v = value @ Wv.T
    2H half-heads of size 32; att = softmax(causal(q k^T / sqrt(32)))
    att = att_half1 - lambda_full * att_half2        (per head)
    out = att @ v ; RMSNorm(head dim) * g * (1-lambda_init) ; out @ Wo.T

Sharding: batch*heads across 8 cores. Core c handles batch b=c//4 and 4
heads hs=4*(c%4)..hs+4. Host sums the 4 per-core partial Wo projections of
each batch.

Per-core layout (all "transposed", partition = feature dim):
  qT/kT   (128, 2048) per head-pair: partitions = 4 half-heads x 32 dims
  sT      scores (n-tile 128, t) -> exp -> pT (fp16)
  v_s     (128 n, 16 nu, 2 head, 65) fp16; col 64 = ones (softmax denom trick)
  oT      (65, t) accumulated in PSUM per (head, half); row 64 = l (denom)
  stats   reciprocal + Ln/Exp rsqrt (same ACT table set as softmax exp),
          per-token scalars broadcast across partitions via DRAM bounce DMA
  o_norm  (128, t) per pair -> y = o_norm^T @ WoT partial, fp32 out
All matmul operands fp16 (fp32 PSUM accumulation); scale 1/sqrt(32) folded
into Wq, g*(1-lambda_init) folded into Wo, lambda applied on-chip to 1/l2.

PSUM budget (8 banks): "sqk" tag 2 bufs x (128,2,512) = 4 banks, shared by
the projection/ms/y accumulators; "oav" 1 buf x (65,4,512) = 4 banks.
"""

import math
from contextlib import ExitStack

import numpy as np

import concourse.bass as bass
import concourse.tile as tile
from concourse import bacc, mybir
from concourse.bass_utils import run_bass_kernel_spmd

# Prefer the combined ln+exp ACT table set so softmax Exp and RMSNorm Ln/Exp
# share one set (a set switch costs ~2.7us on ScalarE).
import concourse.hw_specs as _hw_specs
_orig_get_tables = _hw_specs.get_activation_tables
def _tables_ln_exp_first(arch):
    # Keep dict order (act_func_set_id is the index into this list) but make
    # the exp-only / ln-only sets unselectable so Exp and Ln both resolve to
    # the combined set.
    tabs = _orig_get_tables(arch)
    if "natural_log_exp_and_others" not in tabs:
        return tabs
    return {
        k: (set() if k in ("exp_and_others", "natural_log", "exp_and_friends")
            else v)
        for k, v in tabs.items()
    }
_hw_specs.get_activation_tables = _tables_ln_exp_first
bacc.get_activation_tables = _tables_ln_exp_first

dt = mybir.dt
AF = mybir.ActivationFunctionType

B, T, NN, HID = 2, 2048, 2048, 1024
H, DH, HALF = 16, 64, 32
DEPTH = 6
LAMBDA_INIT = 0.8 - 0.6 * math.exp(-0.3 * DEPTH)
EPS = 1e-5
N_CORES = 8
HPC = 4          # heads per core
NEG = -1.0e30

_CACHE = {}


def _build(lam: float, reps: int = 1):
    nc = bacc.Bacc(
        "TRN2", target_bir_lowering=False, debug=False, num_devices=N_CORES
    )

    f16, f32 = dt.float16, dt.float32

    xq_d = nc.dram_tensor("xq", [HID, T], f16, kind="ExternalInput").ap()
    xk_d = nc.dram_tensor("xk", [HID, T], f16, kind="ExternalInput").ap()
    xv_d = nc.dram_tensor("xv", [HID, T], f16, kind="ExternalInput").ap()
    wq_d = nc.dram_tensor("wq", [HID, 256], f16, kind="ExternalInput").ap()
    wk_d = nc.dram_tensor("wk", [HID, 256], f16, kind="ExternalInput").ap()
    wv_d = nc.dram_tensor("wv", [HID, 256], f16, kind="ExternalInput").ap()
    wo_d = nc.dram_tensor("wo", [256, HID], f16, kind="ExternalInput").ap()
    mask_d = nc.dram_tensor("mask", [128, 128], f16, kind="ExternalInput").ap()
    y_d = nc.dram_tensor("y", [T, HID], f16, kind="ExternalOutput").ap()
    scr_d = nc.dram_tensor("scr", [2, 7, T], f16).ap()  # row bounce buffer

    with tile.TileContext(nc) as tc, ExitStack() as ctx:
        ctx.enter_context(
            nc.allow_low_precision(reason="fp16 attention pipeline by design")
        )
        consts = ctx.enter_context(tc.tile_pool(name="consts", bufs=1))
        xpool = ctx.enter_context(tc.tile_pool(name="xpool", bufs=2))
        qkpool = ctx.enter_context(tc.tile_pool(name="qkpool", bufs=1))
        vpool = ctx.enter_context(tc.tile_pool(name="vpool", bufs=1))
        ppool = ctx.enter_context(tc.tile_pool(name="ppool", bufs=4))
        opool = ctx.enter_context(tc.tile_pool(name="opool", bufs=1))
        npool = ctx.enter_context(tc.tile_pool(name="npool", bufs=1))
        spool = ctx.enter_context(tc.tile_pool(name="spool", bufs=2))
        bpool = ctx.enter_context(tc.tile_pool(name="bpool", bufs=2))
        ypool = ctx.enter_context(tc.tile_pool(name="ypool", bufs=3))
        psum = ctx.enter_context(tc.tile_pool(name="psum", bufs=1, space="PSUM"))

        def sqk_tile(shape, name):
            return psum.tile(shape, f32, tag="sqk", bufs=3, name=name)

        for _rep in range(reps):
            # ---------------- constants ----------------
            wq_s = consts.tile([128, 8, 256], f16, tag="wq")
            wk_s = consts.tile([128, 8, 256], f16, tag="wk")
            wv_s = consts.tile([128, 8, 256], f16, tag="wv")
            nc.sync.dma_start(out=wq_s, in_=wq_d.rearrange("(d p) j -> p d j", p=128))
            nc.sync.dma_start(out=wk_s, in_=wk_d.rearrange("(d p) j -> p d j", p=128))
            nc.sync.dma_start(out=wv_s, in_=wv_d.rearrange("(d p) j -> p d j", p=128))
            wo_s = consts.tile([128, 2, HID], f16, tag="wo")
            nc.sync.dma_start(out=wo_s, in_=wo_d.rearrange("(k p) e -> p k e", p=128))
            mask_s = consts.tile([128, 128], f16, tag="mask")
            nc.sync.dma_start(out=mask_s, in_=mask_d)
            ones64 = consts.tile([64, 1], f16, tag="ones64")
            nc.vector.memset(ones64, 1.0)
            ebias = consts.tile([128, 1], f32, tag="ebias")
            nc.vector.memset(ebias, EPS)

            # ---------------- projections ----------------
            # qT/kT (j-part, t): lhsT = W chunk (128d, 128j), rhs = xT chunk
            qT = [qkpool.tile([128, T], f16, tag=f"qT{pp}", name=f"qT{pp}")
                  for pp in range(2)]
            kT = [qkpool.tile([128, T], f16, tag=f"kT{pp}", name=f"kT{pp}")
                  for pp in range(2)]
            v_s = [vpool.tile([128, 16, 2, 65], f16, tag=f"v{pp}", name=f"v{pp}")
                   for pp in range(2)]

            def load_x(src):
                xt = xpool.tile([128, 8, T], f16, tag="x", name="xt")
                for d in range(8):
                    nc.sync.dma_start(
                        out=xt[:, d, :], in_=src[128 * d : 128 * d + 128, :]
                    )
                return xt

            def qk_group(xt, w_s, dst, pp, c):
                acc = sqk_tile([128, 512], "acc")
                for d in range(8):
                    nc.tensor.matmul(
                        acc,
                        w_s[:, d, 128 * pp : 128 * pp + 128],
                        xt[:, d, 512 * c : 512 * c + 512],
                        start=(d == 0),
                        stop=(d == 7),
                    )
                nc.vector.tensor_copy(dst[pp][:, 512 * c : 512 * c + 512], acc)

            def v_group(xt, pp, nu):
                # v (n-part, j): lhsT = xvT chunk (128d, 128n), rhs = Wv cols
                acc = sqk_tile([128, 128], "accv")
                for d in range(8):
                    nc.tensor.matmul(
                        acc,
                        xt[:, d, 128 * nu : 128 * nu + 128],
                        wv_s[:, d, 128 * pp : 128 * pp + 128],
                        start=(d == 0),
                        stop=(d == 7),
                    )
                nc.vector.tensor_copy(
                    v_s[pp][:, nu, :, 0:64],
                    acc.rearrange("p (g j) -> p g j", g=2),
                )

            # pair-0 projections up front (+ all of q so xq's slot frees for
            # xv); pair-1 k/v groups are drip-fed into attention pair-0 below
            # where the PE has slack under the ACT-bound exp stream.
            xqt = load_x(xq_d)
            for c in range(4):
                qk_group(xqt, wq_s, qT, 0, c)
            for c in range(4):
                qk_group(xqt, wq_s, qT, 1, c)
            xkt = load_x(xk_d)
            for c in range(4):
                qk_group(xkt, wk_s, kT, 0, c)
            nc.vector.memset(v_s[0][:, :, :, 64:65], 1.0)
            nc.vector.memset(v_s[1][:, :, :, 64:65], 1.0)
            xvt = load_x(xv_d)
            for nu in range(16):
                acc = sqk_tile([128, 256], "accv")
                for d in range(8):
                    nc.tensor.matmul(
                        acc,
                        xvt[:, d, 128 * nu : 128 * nu + 128],
                        wv_s[:, d, :],
                        start=(d == 0),
                        stop=(d == 7),
                    )
                for pp in range(2):
                    nc.vector.tensor_copy(
                        v_s[pp][:, nu, :, 0:64],
                        acc.rearrange("p (g j) -> p g j", g=4)[
                            :, 2 * pp : 2 * pp + 2, :
                        ],
                    )

            for c in range(4):
                qk_group(xkt, wk_s, kT, 1, c)

            # ---------------- attention + per-pair stats ----------------
            o_norm = [npool.tile([128, 4, 512], f16, tag=f"on{pp}",
                                 name=f"on{pp}") for pp in range(2)]

            def stats_rows(pp, o_s):
                for g in range(2):
                    nc.sync.dma_start(
                        out=scr_d[pp, 0 + g : 1 + g, :],
                        in_=o_s[64:65, 2 * g, :, :].rearrange(
                            "p a b -> p (a b)"),
                    )
                    nc.sync.dma_start(
                        out=scr_d[pp, 2 + g : 3 + g, :],
                        in_=o_s[64:65, 2 * g + 1, :, :].rearrange(
                            "p a b -> p (a b)"),
                    )

            def stats(pp, o_s):
                # softmax denominators + differential combine + RMSNorm.
                # Ln/Exp live in the same ACT table set as the softmax Exp.
                for g in (1, 0):
                    aB = bpool.tile([64, 4, 512], f16, tag="bcA", name="aB")
                    bB = bpool.tile([64, 4, 512], f16, tag="bcB", name="bB")

                    def bcast(dst, r):
                        src = scr_d[pp, r, :]
                        nc.sync.dma_start(
                            out=dst.rearrange("p a b -> p (a b)"),
                            in_=bass.AP(tensor=src.tensor, offset=src.offset,
                                        ap=[[0, 64]] + src.ap),
                        )

                    bcast(aB, 0 + g)
                    bcast(bB, 2 + g)
                    nc.vector.reciprocal(aB, aB)
                    nc.vector.reciprocal(bB, bB)
                    nc.vector.tensor_scalar_mul(bB, bB, lam)

                    od = spool.tile([64, 4, 512], f16, tag="od", name="od")
                    m2 = spool.tile([64, 4, 512], f16, tag="m2", name="m2",
                                    bufs=1)
                    nc.vector.tensor_mul(od, o_s[0:64, 2 * g, :, :], aB)
                    nc.vector.tensor_mul(m2, o_s[0:64, 2 * g + 1, :, :], bB)
                    nc.vector.tensor_sub(od, od, m2)
                    sq = spool.tile([64, 4, 512], f16, tag="m2", name="sq",
                                    bufs=1)
                    nc.vector.tensor_mul(sq, od, od)

                    crow = spool.tile([1, 4, 512], f16, tag="crow", name="crow",
                                      bufs=1)
                    for c in range(4):
                        pm = sqk_tile([1, 512], "pm")
                        nc.tensor.matmul(pm, ones64, sq[:, c, :], start=True,
                                         stop=True)
                        nc.scalar.activation(
                            out=crow[:, c, :], in_=pm, func=AF.Ln,
                            scale=1.0 / DH, bias=ebias[0:1, :],
                        )
                    nc.scalar.activation(out=crow, in_=crow, func=AF.Exp,
                                         scale=-0.5)
                    nc.sync.dma_start(
                        out=scr_d[pp, 4 + g : 5 + g, :],
                        in_=crow.rearrange("p a b -> p (a b)"),
                    )
                    cB = bpool.tile([64, 4, 512], f16, tag="bcC", name="cB",
                                    bufs=2)
                    bcast(cB, 4 + g)
                    if g == 0:
                        nc.vector.tensor_mul(o_norm[pp][0:64, :, :], od, cB)
                    else:
                        onh = spool.tile([64, 4, 512], f16, tag="onh",
                                         name="onh")
                        nc.vector.tensor_mul(onh, od, cB)
                        nc.sync.dma_start(out=o_norm[pp][64:128, :, :],
                                          in_=onh)

            oss = []
            for pp in range(2):
                o_s = opool.tile([65, 4, 4, 512], f16, tag=f"o_{pp}",
                                 name=f"o_{pp}")
                oss.append(o_s)
                for c in range(4):
                    last = 4 * c + 3
                    for g in range(2):  # head-in-pair
                        po = psum.tile([65, 2, 512], f32, tag="oav", bufs=1,
                                       name="po")
                        for nu in range(last + 1):
                            lo = 128 * (nu - 4 * c) if nu >= 4 * c else 0
                            ps = sqk_tile([128, 2, 512], "ps")
                            for s in range(2):  # half
                                hh = 2 * g + s
                                nc.tensor.matmul(
                                    ps[:, s, lo:512],
                                    kT[pp][32 * hh : 32 * hh + 32,
                                           128 * nu : 128 * nu + 128],
                                    qT[pp][32 * hh : 32 * hh + 32,
                                           512 * c + lo : 512 * c + 512],
                                    start=True,
                                    stop=True,
                                    tile_position=(32 * hh, 0),
                                )
                            pt = ppool.tile([128, 2, 512], f16, tag="pt",
                                            name="pt")
                            nc.scalar.activation(
                                out=pt[:, :, lo:512], in_=ps[:, :, lo:512],
                                func=AF.Exp,
                            )
                            if nu >= 4 * c:  # diagonal: zero masked probs
                                for s in range(2):
                                    nc.vector.tensor_mul(
                                        pt[:, s, lo : lo + 128],
                                        pt[:, s, lo : lo + 128],
                                        mask_s,
                                    )
                            for s in range(2):
                                nc.tensor.matmul(
                                    po[:, s, lo:512],
                                    v_s[pp][:, nu, g, :],
                                    pt[:, s, lo:512],
                                    start=(nu == 0),
                                    stop=(nu == last),
                                )
                        nc.vector.tensor_copy(
                            o_s[:, 2 * g : 2 * g + 2, c, :], po
                        )
                stats_rows(pp, o_s)
            for pp in range(2):
                stats(pp, oss[pp])

            # ---- y = o_norm^T @ WoT (partial; host sums over cores) ----
            # PE warm-up: HAM re-throttles after the PE-idle stats phase;
            # ~3.5us of junk matmuls restore the 2.4 GHz clock before the
            # output projection. A dummy DMA keeps them DCE-alive.
            wpm = sqk_tile([1, 512], "wpm")
            for w in range(8):
                nc.tensor.matmul(wpm, ones64, o_norm[0][0:64, 0, 0:512],
                                 start=True, stop=True)
            wjunk = spool.tile([1, 512], f16, tag="crow", name="wjunk", bufs=1)
            nc.vector.tensor_copy(wjunk, wpm)
            nc.sync.dma_start(out=scr_d[0, 6:7, 0:512], in_=wjunk)
            for tt in range(16):
                py = sqk_tile([128, 1024], "py")
                for e in range(2):
                    for pp in range(2):
                        nc.tensor.matmul(
                            py[:, 512 * e : 512 * e + 512],
                            o_norm[pp][:, tt // 4,
                                       128 * (tt % 4) : 128 * (tt % 4) + 128],
                            wo_s[:, pp, 512 * e : 512 * e + 512],
                            start=(pp == 0),
                            stop=(pp == 1),
                        )
                ys = ypool.tile([128, 1024], f16, tag="ys", name="ys")
                if tt % 2 == 0:
                    nc.vector.tensor_copy(ys, py)
                else:
                    nc.scalar.copy(ys, py)
                nc.sync.dma_start(out=y_d[128 * tt : 128 * tt + 128, :], in_=ys)

    nc.compile()
    return nc


def _prep(inputs):
    a = {k: np.asarray(v) for k, v in inputs.items()}
    lam = float(
        np.exp(np.sum(a["lq1"] * a["lk1"], dtype=np.float32))
        - np.exp(np.sum(a["lq2"] * a["lk2"], dtype=np.float32))
        + LAMBDA_INIT
    )
    wq_t = (a["Wq"].T / math.sqrt(HALF)).astype(np.float16)
    wk_t = a["Wk"].T.astype(np.float16)
    wv_t = a["Wv"].T.astype(np.float16)
    wo_g = (a["Wo"] * (np.tile(a["g"], H) * (1.0 - LAMBDA_INIT))[None, :]).T.astype(
        np.float16
    )
    r = np.arange(128)
    mask = (r[:, None] <= r[None, :]).astype(np.float16)

    in_maps = []
    for core in range(N_CORES):
        b, hs = core // 4, 4 * (core % 4)
        sl = slice(DH * hs, DH * hs + DH * HPC)
        in_maps.append({
            "xq": np.ascontiguousarray(a["query"][b].T).astype(np.float16),
            "xk": np.ascontiguousarray(a["key_"][b].T).astype(np.float16),
            "xv": np.ascontiguousarray(a["value"][b].T).astype(np.float16),
            "wq": np.ascontiguousarray(wq_t[:, sl]),
            "wk": np.ascontiguousarray(wk_t[:, sl]),
            "wv": np.ascontiguousarray(wv_t[:, sl]),
            "wo": np.ascontiguousarray(wo_g[sl, :]),
            "mask": mask,
        })
    return lam, in_maps


def run(inputs, trace=False, reps=1):
    lam, in_maps = _prep(inputs)
    key = (round(lam, 6), reps)
    if key not in _CACHE:
        _CACHE[key] = _build(lam, reps)
    nc = _CACHE[key]
    res = run_bass_kernel_spmd(
        nc, in_maps, core_ids=list(range(N_CORES)), trace=trace
    )
    out = np.empty((B, T, HID), np.float32)
    for b in range(B):
        out[b] = sum(res.results[4 * b + i]["y"].astype(np.float32) for i in range(4))
    return out, res


def kernel(**inputs) -> np.ndarray:
    out, _ = run(inputs)
    return out



# revision 39
# speedup vs baseline: 1.1401x; 1.1401x over previous
"""Differential multi-head attention on 8 TRN2 NeuronCores (Bass/Tile).

Problem (hardcoded): B=2, T=N=2048, HID=1024, H=16 heads, DH=64, HALF=32,
DEPTH=6, causal. Reference:
    q = query @ Wq.T ; k = key_ @ Wk.T ; v = value @ Wv.T
    2H half-heads of size 32; att = softmax(causal(q k^T / sqrt(32)))
    att = att_half1 - lambda_full * att_half2        (per head)
    out = att @ v ; RMSNorm(head dim) * g * (1-lambda_init) ; out @ Wo.T

Sharding: batch*heads across 8 cores. Core c handles batch b=c//4 and 4
heads hs=4*(c%4)..hs+4. Host sums the 4 per-core partial Wo projections of
each batch.

Schedule: c-major software pipeline over 512-token query chunks. Per chunk:
DMA loads for the next chunk, q/k/v projections, last chunk's output
projection, then attention for both head-pairs, then per-(pair, chunk)
stats. The exp stream on the ACT engine is the long pole; projections and
output matmuls fill PE slack underneath it; the causal-diagonal mask
multiplies run on the otherwise-idle GPSIMD (Pool) engine; broadcast of
per-token scalars across partitions uses gpsimd.partition_broadcast
(no DRAM bounce).

Per-core layout (all "transposed", partition = feature dim):
  qT/kT   (128, 2048) per head-pair: partitions = 4 half-heads x 32 dims
  sT      scores (n-tile 128, t) -> exp -> pT (fp16)
  v_s     (128 n, 16 nu, 2 head, 65) fp16; col 64 = ones (softmax denom)
  po      (65, 2, 512) PSUM accumulated per (head, half); row 64 = l
  o_norm  (128, t-chunked): g=0 on partitions 0:64, g=1 moved to 64:128 by
          a small SBUF->SBUF DMA; y = o_norm^T @ WoT partial per chunk.
All matmul operands fp16 (fp32 PSUM accumulation); scale 1/sqrt(32) folded
into Wq, g*(1-lambda_init) folded into Wo, lambda applied on-chip to 1/l2.

PSUM budget (8 banks): "ring" tag 2 bufs x 4KB (proj acc / scores ps /
rowsum pm / output py) = 4 banks; "oav" tag 2 bufs x (65,2,512) = 4 banks.
"""

import math
from contextlib import ExitStack

import numpy as np

import concourse.bass as bass
import concourse.bass_isa as bass_isa
import concourse.tile as tile
from concourse import bacc, mybir
from concourse.bass_utils import run_bass_kernel_spmd

# Prefer the combined ln+exp ACT table set so softmax Exp and RMSNorm Ln/Exp
# share one set (a set switch costs ~2.7us on ScalarE).
import concourse.hw_specs as _hw_specs
_orig_get_tables = _hw_specs.get_activation_tables
def _tables_ln_exp_first(arch):
    tabs = _orig_get_tables(arch)
    if "natural_log_exp_and_others" not in tabs:
        return tabs
    return {
        k: (set() if k in ("exp_and_others", "natural_log", "exp_and_friends")
            else v)
        for k, v in tabs.items()
    }
_hw_specs.get_activation_tables = _tables_ln_exp_first
bacc.get_activation_tables = _tables_ln_exp_first

dt = mybir.dt
AF = mybir.ActivationFunctionType
AL = mybir.AluOpType

B, T, NN, HID = 2, 2048, 2048, 1024
H, DH, HALF = 16, 64, 32
DEPTH = 6
LAMBDA_INIT = 0.8 - 0.6 * math.exp(-0.3 * DEPTH)
EPS = 1e-5
N_CORES = 8
HPC = 4          # heads per core

_CACHE = {}


def _build(lam: float, reps: int = 1):
    nc = bacc.Bacc(
        "TRN2", target_bir_lowering=False, debug=False, num_devices=N_CORES
    )

    f16, f32 = dt.float16, dt.float32

    xq_d = nc.dram_tensor("xq", [HID, T], f16, kind="ExternalInput").ap()
    xk_d = nc.dram_tensor("xk", [HID, T], f16, kind="ExternalInput").ap()
    xv_d = nc.dram_tensor("xv", [HID, T], f16, kind="ExternalInput").ap()
    wq_d = nc.dram_tensor("wq", [HID, 256], f16, kind="ExternalInput").ap()
    wk_d = nc.dram_tensor("wk", [HID, 256], f16, kind="ExternalInput").ap()
    wv_d = nc.dram_tensor("wv", [HID, 256], f16, kind="ExternalInput").ap()
    wo_d = nc.dram_tensor("wo", [256, HID], f16, kind="ExternalInput").ap()
    mask_d = nc.dram_tensor("mask", [128, 128], f16, kind="ExternalInput").ap()
    y_d = nc.dram_tensor("y", [T, HID], f16, kind="ExternalOutput").ap()

    xq_r = xq_d.rearrange("(d p) t -> p d t", p=128)
    xk_r = xk_d.rearrange("(d p) t -> p d t", p=128)
    xv_r = xv_d.rearrange("(d p) t -> p d t", p=128)

    with tile.TileContext(nc) as tc, ExitStack() as ctx:
        ctx.enter_context(
            nc.allow_low_precision(reason="fp16 attention pipeline by design")
        )
        consts = ctx.enter_context(tc.tile_pool(name="consts", bufs=1))
        xpool = ctx.enter_context(tc.tile_pool(name="xpool", bufs=2))
        qkpool = ctx.enter_context(tc.tile_pool(name="qkpool", bufs=1))
        vpool = ctx.enter_context(tc.tile_pool(name="vpool", bufs=1))
        ppool = ctx.enter_context(tc.tile_pool(name="ppool", bufs=4))
        opool = ctx.enter_context(tc.tile_pool(name="opool", bufs=1))
        npool = ctx.enter_context(tc.tile_pool(name="npool", bufs=1))
        spool = ctx.enter_context(tc.tile_pool(name="spool", bufs=2))
        ypool = ctx.enter_context(tc.tile_pool(name="ypool", bufs=3))
        psum = ctx.enter_context(tc.tile_pool(name="psum", bufs=1, space="PSUM"))

        def ring(shape, name):
            return psum.tile(shape, f32, tag="ring", bufs=2, name=name)

        for _rep in range(reps):
            # ---------------- constants ----------------
            wq_s = consts.tile([128, 8, 256], f16, tag="wq")
            wk_s = consts.tile([128, 8, 256], f16, tag="wk")
            wv_s = consts.tile([128, 8, 256], f16, tag="wv")
            wo_s = consts.tile([128, 2, HID], f16, tag="wo")
            mask_s = consts.tile([128, 128], f16, tag="mask")
            ones64 = consts.tile([64, 1], f16, tag="ones64")
            nc.vector.memset(ones64, 1.0)
            ebias = consts.tile([128, 1], f32, tag="ebias")
            nc.vector.memset(ebias, EPS)

            qT = [qkpool.tile([128, T], f16, tag=f"qT{pp}", name=f"qT{pp}")
                  for pp in range(2)]
            kT = [qkpool.tile([128, T], f16, tag=f"kT{pp}", name=f"kT{pp}")
                  for pp in range(2)]
            v_s = [vpool.tile([128, 16, 2, 65], f16, tag=f"v{pp}", name=f"v{pp}")
                   for pp in range(2)]
            nc.vector.memset(v_s[0][:, :, :, 64:65], 1.0)
            nc.vector.memset(v_s[1][:, :, :, 64:65], 1.0)
            o_s = [opool.tile([65, 4, 4, 512], f16, tag=f"o_{pp}",
                              name=f"o_{pp}") for pp in range(2)]
            o_norm = [npool.tile([128, 4, 512], f16, tag=f"on{pp}",
                                 name=f"on{pp}") for pp in range(2)]

            # DMA emission order matters: one SP queue. Weights for q/k
            # first, then chunk-0 x slabs, then the rest.
            nc.sync.dma_start(out=wq_s, in_=wq_d.rearrange("(d p) j -> p d j", p=128))
            nc.sync.dma_start(out=wk_s, in_=wk_d.rearrange("(d p) j -> p d j", p=128))

            xt = {}  # (name, c) -> tile

            def load_chunk(c):
                for nm, src in (("xq", xq_r), ("xk", xk_r), ("xv", xv_r)):
                    t = xpool.tile([128, 8, 512], f16, tag=nm, bufs=2,
                                   name=f"{nm}{c}")
                    nc.sync.dma_start(out=t, in_=src[:, :, 512 * c:512 * c + 512])
                    xt[(nm, c)] = t

            load_chunk(0)
            nc.sync.dma_start(out=wv_s, in_=wv_d.rearrange("(d p) j -> p d j", p=128))
            nc.sync.dma_start(out=mask_s, in_=mask_d)
            nc.sync.dma_start(out=wo_s, in_=wo_d.rearrange("(k p) e -> p k e", p=128))
            # pair-1 g1 rows of Wo on partitions 0:64, for the split output
            # projection of the last chunk (avoids the lane-move DMA on the
            # critical tail)
            wo_lo1 = consts.tile([64, HID], f16, tag="wo_lo1")
            nc.sync.dma_start(out=wo_lo1, in_=wo_d[192:256, :])

            # ---------------- building blocks ----------------
            def qk_group(xtile, w_s, dst, pp, c):
                acc = psum.tile([128, 512], f32, tag="oav", bufs=2,
                                name="acc")
                for d in range(8):
                    nc.tensor.matmul(
                        acc,
                        w_s[:, d, 128 * pp:128 * pp + 128],
                        xtile[:, d, :],
                        start=(d == 0),
                        stop=(d == 7),
                    )
                nc.vector.tensor_copy(dst[pp][:, 512 * c:512 * c + 512], acc)

            def v_group(xtile, nu, tag="oav"):
                accv = psum.tile([128, 256], f32, tag=tag, bufs=2,
                                 name="accv")
                for d in range(8):
                    nc.tensor.matmul(
                        accv,
                        xtile[:, d, 128 * (nu % 4):128 * (nu % 4) + 128],
                        wv_s[:, d, :],
                        start=(d == 0),
                        stop=(d == 7),
                    )
                for pp in range(2):
                    nc.vector.tensor_copy(
                        v_s[pp][:, nu, :, 0:64],
                        accv.rearrange("p (g j) -> p g j", g=4)[
                            :, 2 * pp:2 * pp + 2, :
                        ],
                    )

            # mask viewed as (128, 2, 128) via a 0-stride middle dim so one
            # DVE op masks both halves of a diagonal tile
            def mask2():
                return bass.AP(
                    tensor=mask_s.tensor, offset=mask_s.offset,
                    ap=[mask_s.ap[0], [0, 2], mask_s.ap[1]],
                )

            def attention(c, pp, sched, mid=None, pre_av=None):
                last = 4 * c + 3
                for g in range(2):  # head-in-pair
                    po = psum.tile([65, 2, 512], f32, tag="oav", bufs=2,
                                   name="po")
                    for nu in range(last + 1):
                        lo = 128 * (nu - 4 * c) if nu >= 4 * c else 0
                        ps = ring([128, 2, 512], "ps")
                        for s in range(2):  # half
                            hh = 2 * g + s
                            nc.tensor.matmul(
                                ps[:, s, lo:512],
                                kT[pp][32 * hh:32 * hh + 32,
                                       128 * nu:128 * nu + 128],
                                qT[pp][32 * hh:32 * hh + 32,
                                       512 * c + lo:512 * c + 512],
                                start=True,
                                stop=True,
                                tile_position=(32 * hh, 0),
                            )
                        pt = ppool.tile([128, 2, 512], f16, tag="pt",
                                        name="pt")
                        nc.scalar.activation(
                            out=pt[:, :, lo:512], in_=ps[:, :, lo:512],
                            func=AF.Exp,
                        )
                        if nu >= 4 * c:  # diagonal: zero masked probs
                            nc.vector.tensor_mul(
                                pt[:, :, lo:lo + 128],
                                pt[:, :, lo:lo + 128],
                                mask2(),
                            )
                        if pre_av is not None:
                            pre_av(g, nu)
                        for s in range(2):
                            nc.tensor.matmul(
                                po[:, s, lo:512],
                                v_s[pp][:, nu, g, :],
                                pt[:, s, lo:512],
                                start=(nu == 0),
                                stop=(nu == last),
                            )
                    nc.vector.tensor_copy(o_s[pp][:, 2 * g:2 * g + 2, c, :], po)
                    if g == 0 and mid is not None:
                        mid()
                    for th in sched.get((pp, g), ()):
                        th()

            st = {}  # pp -> (od, ms) tiles awaiting stats_act

            def stats_pre(pp, c):
                # Per-token scalars for chunk c of pair pp (DVE/Pool only —
                # no ACT work, so the exp stream is never blocked behind it).
                # l-rows live at o_s[64, 2g+s, c, :].
                rl = spool.tile([1, 4, 512], f16, tag="rl", name="rl")
                nc.vector.reciprocal(rl, o_s[pp][64:65, :, c, :])
                rB = spool.tile([64, 4, 512], f16, tag="rB", name="rB")
                nc.gpsimd.partition_broadcast(rB, rl)

                od = spool.tile([64, 2, 512], f16, tag="od", name="od")
                for g in range(2):
                    m1 = spool.tile([64, 512], f16, tag="m1", name="m1")
                    m2 = spool.tile([64, 512], f16, tag="m2", name="m2")
                    nc.vector.tensor_mul(
                        m1, o_s[pp][0:64, 2 * g, c, :], rB[:, 2 * g, :]
                    )
                    nc.vector.scalar_tensor_tensor(
                        out=m2, in0=o_s[pp][0:64, 2 * g + 1, c, :], scalar=lam,
                        in1=rB[:, 2 * g + 1, :], op0=AL.mult, op1=AL.mult,
                    )
                    nc.vector.tensor_sub(od[:, g, :], m1, m2)
                sq = spool.tile([64, 2, 512], f16, tag="sq", name="sq")
                nc.vector.tensor_mul(sq, od, od)
                ms = spool.tile([64, 2, 512], f32, tag="ms", name="ms")
                nc.gpsimd.partition_all_reduce(ms, sq, 64,
                                               bass_isa.ReduceOp.add)
                st[pp] = (od, ms)

            def stats_g(pp, c, g, lane_dma=True):
                # single-head stats, used to shorten the tail of the last
                # chunk: full chain per g so it pipelines under attention
                rl = spool.tile([1, 2, 512], f16, tag="rlg", name="rlg")
                nc.vector.reciprocal(rl, o_s[pp][64:65, 2 * g:2 * g + 2, c, :])
                rB = spool.tile([64, 2, 512], f16, tag="rBg", name="rBg")
                nc.gpsimd.partition_broadcast(rB, rl)
                m1 = spool.tile([64, 512], f16, tag="m1", name="m1")
                m2 = spool.tile([64, 512], f16, tag="m2", name="m2")
                nc.vector.tensor_mul(m1, o_s[pp][0:64, 2 * g, c, :], rB[:, 0, :])
                nc.vector.scalar_tensor_tensor(
                    out=m2, in0=o_s[pp][0:64, 2 * g + 1, c, :], scalar=lam,
                    in1=rB[:, 1, :], op0=AL.mult, op1=AL.mult,
                )
                odg = spool.tile([64, 512], f16, tag="odg", name="odg")
                nc.vector.tensor_sub(odg, m1, m2)
                sqg = spool.tile([64, 512], f16, tag="sqg", name="sqg")
                nc.vector.tensor_mul(sqg, odg, odg)
                msg = spool.tile([64, 512], f32, tag="msg", name="msg")
                nc.gpsimd.partition_all_reduce(msg, sqg, 64,
                                               bass_isa.ReduceOp.add)
                rsg = spool.tile([64, 512], f16, tag="rsg", name="rsg")
                nc.scalar.activation(out=rsg, in_=msg, func=AF.Ln,
                                     scale=1.0 / DH, bias=ebias[0:64, :])
                nc.scalar.activation(out=rsg, in_=rsg, func=AF.Exp,
                                     scale=-0.5)
                if g == 0:
                    nc.vector.tensor_mul(o_norm[pp][0:64, c, :], odg, rsg)
                    return None
                onh = spool.tile([64, 512], f16, tag="onh", name="onh")
                nc.vector.tensor_mul(onh, odg, rsg)
                if lane_dma:
                    nc.sync.dma_start(out=o_norm[pp][64:128, c, :], in_=onh)
                return onh

            def stats_act(pp, c):
                od, ms = st[pp]
                rsB = spool.tile([64, 2, 512], f16, tag="rsB", name="rsB")
                nc.scalar.activation(out=rsB, in_=ms, func=AF.Ln,
                                     scale=1.0 / DH, bias=ebias[0:64, :])
                nc.scalar.activation(out=rsB, in_=rsB, func=AF.Exp,
                                     scale=-0.5)
                nc.vector.tensor_mul(o_norm[pp][0:64, c, :], od[:, 0, :],
                                     rsB[:, 0, :])
                onh = spool.tile([64, 512], f16, tag="onh", name="onh")
                nc.vector.tensor_mul(onh, od[:, 1, :], rsB[:, 1, :])
                nc.sync.dma_start(out=o_norm[pp][64:128, c, :], in_=onh)

            def y_block(c):
                for tt in range(4 * c, 4 * c + 4):
                    py = psum.tile([128, 1024], f32, tag="oav", bufs=2,
                                   name="py")
                    for e in range(2):
                        for pp in range(2):
                            nc.tensor.matmul(
                                py[:, 512 * e:512 * e + 512],
                                o_norm[pp][:, tt // 4,
                                           128 * (tt % 4):128 * (tt % 4) + 128],
                                wo_s[:, pp, 512 * e:512 * e + 512],
                                start=(pp == 0),
                                stop=(pp == 1),
                            )
                    ys = ypool.tile([128, 1024], f16, tag="ys", name="ys")
                    nc.vector.tensor_copy(ys, py)
                    nc.sync.dma_start(out=y_d[128 * tt:128 * tt + 128, :],
                                      in_=ys)

            def y_split(c, onh1):
                # last chunk: pair-1 g1 contribution comes straight from the
                # onh staging tile (partitions 0:64) instead of waiting for
                # the lane-move DMA. Paged: the independent matmuls of the
                # next tt run ahead while the previous tt waits on onh1, so
                # the PE stays warm through the stats chain.
                pys = {}

                def part(tt):
                    py = psum.tile([128, 1024], f32, tag="oav", bufs=2,
                                   name="py")
                    pys[tt] = py
                    sl = slice(128 * (tt % 4), 128 * (tt % 4) + 128)
                    for e in range(2):
                        esl = slice(512 * e, 512 * e + 512)
                        nc.tensor.matmul(py[:, esl],
                                         o_norm[0][:, tt // 4, sl],
                                         wo_s[:, 0, esl],
                                         start=True, stop=False)
                        nc.tensor.matmul(py[:, esl],
                                         o_norm[1][0:64, tt // 4, sl],
                                         wo_s[0:64, 1, esl],
                                         start=False, stop=False)

                def fin(tt):
                    py = pys[tt]
                    sl = slice(128 * (tt % 4), 128 * (tt % 4) + 128)
                    for e in range(2):
                        esl = slice(512 * e, 512 * e + 512)
                        nc.tensor.matmul(py[:, esl], onh1[:, sl],
                                         wo_lo1[:, esl],
                                         start=False, stop=True)
                    ys = ypool.tile([128, 1024], f16, tag="ys", name="ys")
                    nc.vector.tensor_copy(ys, py)
                    nc.sync.dma_start(out=y_d[128 * tt:128 * tt + 128, :],
                                      in_=ys)

                t0 = 4 * c
                part(t0); part(t0 + 1); fin(t0); part(t0 + 2)
                fin(t0 + 1); part(t0 + 3); fin(t0 + 2); fin(t0 + 3)

            # ---------------- main pipeline ----------------
            def qk_thunk(c, which, pp):
                src, w, dst = (("xq", wq_s, qT) if which == "q"
                               else ("xk", wk_s, kT))
                return lambda: qk_group(xt[(src, c)], w, dst, pp, c)

            def v_thunk(c, nu):
                return lambda: v_group(xt[("xv", c)], nu)

            def y_thunk(tt):
                def th():
                    py = psum.tile([128, 1024], f32, tag="oav", bufs=2,
                                   name="py")
                    for e in range(2):
                        for pp in range(2):
                            nc.tensor.matmul(
                                py[:, 512 * e:512 * e + 512],
                                o_norm[pp][:, tt // 4,
                                           128 * (tt % 4):128 * (tt % 4) + 128],
                                wo_s[:, pp, 512 * e:512 * e + 512],
                                start=(pp == 0),
                                stop=(pp == 1),
                            )
                    ys = ypool.tile([128, 1024], f16, tag="ys", name="ys")
                    nc.vector.tensor_copy(ys, py)
                    nc.sync.dma_start(out=y_d[128 * tt:128 * tt + 128, :],
                                      in_=ys)
                return th

            # prologue: only q/k of pair 0 for chunk 0; chunk-0 v-groups are
            # dripped inside attention(0,0) right before the av that needs
            # them (ps-ring accumulators keep the oav ring cycle-free)
            qk_group(xt[("xq", 0)], wq_s, qT, 0, 0)
            qk_group(xt[("xk", 0)], wk_s, kT, 0, 0)

            def pre_av0(g, nu):
                if g == 0:
                    v_group(xt[("xv", 0)], nu, tag="ring")

            pend = None
            for c in range(4):
                if c < 3:
                    load_chunk(c + 1)
                    p = ([qk_thunk(c + 1, "q", 0), qk_thunk(c + 1, "k", 0),
                          qk_thunk(c + 1, "q", 1), qk_thunk(c + 1, "k", 1)]
                         + [v_thunk(c + 1, nu)
                            for nu in range(4 * c + 4, 4 * c + 8)])
                else:
                    p = []
                yth = [y_thunk(tt) for tt in range(4 * (c - 1), 4 * c)] \
                    if 0 < c < 3 else []
                if c == 0:
                    # pair-1 q/k for chunk 0 first, then chunk-1 work
                    p = [qk_thunk(0, "q", 1), qk_thunk(0, "k", 1)] + p
                    sched = {(0, 0): p[0:2], (0, 1): p[2:5],
                             (1, 0): p[5:8], (1, 1): p[8:10]}
                elif c < 3:
                    sched = {(0, 0): p[0:2], (0, 1): [yth[0]] + p[2:4],
                             (1, 0): [yth[1]] + p[4:6],
                             (1, 1): [yth[2], yth[3]] + p[6:8]}
                else:
                    yth = [y_thunk(tt) for tt in range(8, 12)]
                    sched = {(0, 0): [], (0, 1): yth[0:2],
                             (1, 0): yth[2:4], (1, 1): []}
                attention(c, 0, sched, mid=pend if c > 0 else None,
                          pre_av=pre_av0 if c == 0 else None)
                stats_pre(0, c)
                if c < 3:
                    attention(c, 1, sched)
                    stats_pre(1, c)
                    stats_act(0, c)
                    pend = (lambda c=c: stats_act(1, c))
                else:
                    # tail: per-head stats pipelined into the last attention,
                    # then the split output projection
                    attention(c, 1, sched,
                              mid=lambda: (stats_act(0, 3), stats_g(1, 3, 0)))
                    onh13 = stats_g(1, 3, 1, lane_dma=False)
                    y_split(3, onh13)

    nc.compile()
    return nc


def _prep(inputs):
    a = {k: np.asarray(v) for k, v in inputs.items()}
    lam = float(
        np.exp(np.sum(a["lq1"] * a["lk1"], dtype=np.float32))
        - np.exp(np.sum(a["lq2"] * a["lk2"], dtype=np.float32))
        + LAMBDA_INIT
    )
    wq_t = (a["Wq"].T / math.sqrt(HALF)).astype(np.float16)
    wk_t = a["Wk"].T.astype(np.float16)
    wv_t = a["Wv"].T.astype(np.float16)
    wo_g = (a["Wo"] * (np.tile(a["g"], H) * (1.0 - LAMBDA_INIT))[None, :]).T.astype(
        np.float16
    )
    r = np.arange(128)
    mask = (r[:, None] <= r[None, :]).astype(np.float16)

    in_maps = []
    for core in range(N_CORES):
        b, hs = core // 4, 4 * (core % 4)
        sl = slice(DH * hs, DH * hs + DH * HPC)
        in_maps.append({
            "xq": np.ascontiguousarray(a["query"][b].T).astype(np.float16),
            "xk": np.ascontiguousarray(a["key_"][b].T).astype(np.float16),
            "xv": np.ascontiguousarray(a["value"][b].T).astype(np.float16),
            "wq": np.ascontiguousarray(wq_t[:, sl]),
            "wk": np.ascontiguousarray(wk_t[:, sl]),
            "wv": np.ascontiguousarray(wv_t[:, sl]),
            "wo": np.ascontiguousarray(wo_g[sl, :]),
            "mask": mask,
        })
    return lam, in_maps


def run(inputs, trace=False, reps=1):
    lam, in_maps = _prep(inputs)
    key = (round(lam, 6), reps)
    if key not in _CACHE:
        _CACHE[key] = _build(lam, reps)
    nc = _CACHE[key]
    res = run_bass_kernel_spmd(
        nc, in_maps, core_ids=list(range(N_CORES)), trace=trace
    )
    out = np.empty((B, T, HID), np.float32)
    for b in range(B):
        out[b] = sum(res.results[4 * b + i]["y"].astype(np.float32) for i in range(4))
    return out, res


def kernel(**inputs) -> np.ndarray:
    out, _ = run(inputs)
    return out
